# revision 12
# baseline (speedup 1.0000x reference)
"""DOMTransformer Trainium2 kernel.

Strategy (8 NeuronCores, SPMD single NEFF):
  - Pulses are sharded across cores by whole DOMs (block-diagonal attention
    never crosses cores). Within a core, DOMs are packed into 640 slots
    (5 tiles of 128) such that no DOM crosses a 128-slot tile boundary, so
    attention reduces to 5 independent 128x128 block-diagonal tiles.
  - All LayerNorm affines (gamma/beta), residual adds and biases are folded
    host-side into the PE matmul accumulations (diag(gamma) as an extra
    contraction block, biases as rank-1 ones x bias_row matmuls).
  - Attention is computed in S^T = K^T.T @ Q^T orientation; softmax sums come
    from an appended ones-column in the AV matmul; normalization happens on the
    natural [q, head*33] AV output with per-partition tensor_scalar ops.
  - Segment-mean is a pooling matmul with a host-built 1/count matrix;
    empty DOMs stay zero host-side.
"""
import os
import numpy as np

N_CORES = 8
N, PF, D, H, HD, L, DFF, NDOM = 4096, 4, 256, 8, 32, 4, 1024, 512
NPAD = 640
NT = NPAD // 128          # 5 tiles
DOMCAP = 128
EPS = 1e-5
SCALE = float(1.0 / np.sqrt(np.float32(HD)))

# ----------------------------------------------------------------- host side


def _pack_cores(pulse_to_dom_idx):
    ids = np.asarray(pulse_to_dom_idx).astype(np.int64)
    n = ids.shape[0]
    counts = np.bincount(ids, minlength=NDOM)
    doms = np.nonzero(counts)[0]
    dcounts = counts[doms]
    assert dcounts.max() <= 128, "dom larger than a tile is unsupported"
    starts = np.zeros(NDOM + 1, np.int64)
    starts[1:] = np.cumsum(counts)

    def packed_size(cnts):
        off = 0
        for k in cnts:
            if (off % 128) + k > 128:
                off = (off // 128 + 1) * 128
            off += k
        return off

    cores = []
    i = 0
    pulses_left = n
    for c in range(N_CORES):
        cores_left = N_CORES - c
        target = pulses_left / cores_left
        grp = []
        grp_pulses = 0
        while i < len(doms):
            k = dcounts[i]
            if grp and c < N_CORES - 1 and grp_pulses + k > target + 4:
                break
            if packed_size([dcounts[j] for j in range(i - len(grp), i + 1)]) > NPAD:
                break
            grp.append(i)
            grp_pulses += k
            i += 1
        cores.append(grp)
        pulses_left -= grp_pulses
    assert i == len(doms), f"packing overflow: {len(doms) - i} doms left over"

    out = []
    for grp in cores:
        slot_dom = np.full(NPAD, -1, np.int64)
        slot_pulse = np.full(NPAD, -1, np.int64)
        local_doms = []
        off = 0
        for gi in grp:
            d = doms[gi]
            k = dcounts[gi]
            if (off % 128) + k > 128:
                off = (off // 128 + 1) * 128
            slot_dom[off:off + k] = d
            slot_pulse[off:off + k] = np.arange(starts[d], starts[d] + k)
            local_doms.append(d)
            off += k
        assert len(local_doms) <= DOMCAP
        out.append(dict(slot_dom=slot_dom, slot_pulse=slot_pulse,
                        local_doms=np.array(local_doms, np.int64)))
    return out


def _core_inputs(core, pulse_features, counts):
    sd = core["slot_dom"].copy()
    sp = core["slot_pulse"]
    pad = sd < 0
    sd[pad] = NDOM + np.arange(NPAD)[pad]      # unique ids: pads attend to self

    feats = np.zeros((NPAD, PF), np.float32)
    feats[~pad] = np.asarray(pulse_features, np.float32)[sp[~pad]]
    featsT = np.ascontiguousarray(feats.T)

    maskrep = np.zeros((NT, 128, 512), np.float32)
    for t in range(NT):
        s = sd[t * 128:(t + 1) * 128]
        m = (s[:, None] == s[None, :]).astype(np.float32)
        maskrep[t] = np.tile(m, (1, 4))

    poolT = np.zeros((NT, 128, DOMCAP), np.float32)
    for li, d in enumerate(core["local_doms"]):
        sl = np.nonzero(core["slot_dom"] == d)[0]
        w = 1.0 / max(counts[d], 1)
        poolT[sl // 128, sl % 128, li] = w
    return featsT, maskrep, poolT


def _fold_weights(ins):
    f32 = lambda x: np.asarray(x, np.float32)
    Win, b_in = f32(ins["Win"]), f32(ins["b_in"])
    Wqkv, bqkv = f32(ins["Wqkv"]), f32(ins["bqkv"])
    Wout, bout = f32(ins["Wout"]), f32(ins["bout"])
    W1, b1 = f32(ins["W1"]), f32(ins["b1"])
    W2, b2 = f32(ins["W2"]), f32(ins["b2"])
    g1, be1 = f32(ins["g1"]), f32(ins["be1"])
    g2, be2 = f32(ins["g2"]), f32(ins["be2"])

    c = lambda a: np.ascontiguousarray(a, np.float32)
    W = {"win": c(Win)}
    for nm in ("wqk", "bqk", "wv", "bv", "wout", "dprev", "bo2",
               "w1", "b1f", "w2", "d1", "bf2"):
        W[nm] = []
    for l in range(L):
        gp = np.ones(D, np.float32) if l == 0 else g2[l - 1]
        bp = b_in if l == 0 else be2[l - 1]
        Wf = Wqkv[l]
        W["wqk"].append(c((gp[:, None] * Wf[:, :2 * D]).reshape(2, 128, 512)))
        W["bqk"].append(c((bp @ Wf[:, :2 * D] + bqkv[l][:2 * D]).reshape(1, 512)))
        W["wv"].append(c((gp[:, None] * Wf[:, 2 * D:]).reshape(2, 128, 256)))
        W["bv"].append(c((bp @ Wf[:, 2 * D:] + bqkv[l][2 * D:]).reshape(1, 256)))
        W["wout"].append(c(Wout[l].reshape(2, 128, 256)))
        W["dprev"].append(c(np.diag(gp).reshape(2, 128, 256)))
        W["bo2"].append(c((bout[l] + bp).reshape(1, 256)))
        W["w1"].append(c((g1[l][:, None] * W1[l]).reshape(2, 128, 1024)))
        W["b1f"].append(c((be1[l] @ W1[l] + b1[l]).reshape(1, 1024)))
        W["w2"].append(c(W2[l].reshape(8, 128, 256)))
        W["d1"].append(c(np.diag(g1[l]).reshape(2, 128, 256)))
        W["bf2"].append(c((b2[l] + be1[l]).reshape(1, 256)))
    for nm in ("wqk", "bqk", "wv", "bv", "wout", "dprev", "bo2",
               "w1", "b1f", "w2", "d1", "bf2"):
        W[nm] = np.stack(W[nm])
    W["dlast"] = c(np.diag(g2[L - 1]).reshape(2, 128, 256))
    W["blast"] = c(f32(ins["be2"])[L - 1].reshape(1, 256))
    W["gfb"] = c(np.tile(f32(ins["gF"])[None, :], (128, 1)))
    W["bfrow"] = c(f32(ins["bF"]).reshape(1, 256))
    return W


# ------------------------------------------------------------- device program

_CACHE = {}


def _build_program():
    if "nc" in _CACHE:
        return _CACHE["nc"]
    from contextlib import ExitStack
    import concourse.bass as bass
    import concourse.tile as tile
    import concourse.bacc as bacc
    import concourse.mybir as mybir

    dt = mybir.dt.float32
    AF = mybir.ActivationFunctionType
    ALU = mybir.AluOpType

    nc = bacc.Bacc("TRN2", target_bir_lowering=False, debug=False,
                   num_devices=N_CORES)

    def din(name, shape):
        return nc.dram_tensor(name, shape, dt, kind="ExternalInput").ap()

    featsT_d = din("featsT", (PF, NPAD))
    mask_d = din("maskrep", (NT, 128, 512))
    poolT_d = din("poolT", (NT, 128, DOMCAP))
    ident_d = din("ident", (128, 128))
    win_d = din("win", (PF, D))
    wqk_d = din("wqk", (L, 2, 128, 512))
    bqk_d = din("bqk", (L, 1, 512))
    wv_d = din("wv", (L, 2, 128, 256))
    bv_d = din("bv", (L, 1, 256))
    wout_d = din("wout", (L, 2, 128, 256))
    dprev_d = din("dprev", (L, 2, 128, 256))
    bo2_d = din("bo2", (L, 1, 256))
    w1_d = din("w1", (L, 2, 128, 1024))
    b1f_d = din("b1f", (L, 1, 1024))
    w2_d = din("w2", (L, 8, 128, 256))
    d1_d = din("d1", (L, 2, 128, 256))
    bf2_d = din("bf2", (L, 1, 256))
    dlast_d = din("dlast", (2, 128, 256))
    blast_d = din("blast", (1, 256))
    gfb_d = din("gfb", (128, 256))
    bfrow_d = din("bfrow", (1, 256))
    rowsum_d = din("rowsum", (1, DOMCAP))
    out_d = nc.dram_tensor("out", (DOMCAP, D), dt, kind="ExternalOutput").ap()

    with tile.TileContext(nc) as tc, ExitStack() as ctx:
        const = ctx.enter_context(tc.tile_pool(name="const", bufs=1))
        wpool = ctx.enter_context(tc.tile_pool(name="w", bufs=2))
        wpool1 = ctx.enter_context(tc.tile_pool(name="w1p", bufs=1))
        act = ctx.enter_context(tc.tile_pool(name="act", bufs=2))
        big = ctx.enter_context(tc.tile_pool(name="big", bufs=1))
        stat = ctx.enter_context(tc.tile_pool(name="stat", bufs=8))
        ps_a = ctx.enter_context(tc.tile_pool(name="ps_a", bufs=2, space="PSUM"))
        ps_b = ctx.enter_context(tc.tile_pool(name="ps_b", bufs=1, space="PSUM"))
        ps_s = ctx.enter_context(tc.tile_pool(name="ps_s", bufs=2, space="PSUM"))

        MM = nc.tensor.matmul
        _cp = [0]

        def copy(dst, src):
            # alternate PSUM->SBUF copies between DVE and ACT
            if _cp[0] % 2 == 0:
                nc.vector.tensor_copy(dst, src)
            else:
                nc.scalar.copy(dst, src)
            _cp[0] += 1

        # ---- constants
        featsT_sb = const.tile([PF, NPAD], dt)
        nc.sync.dma_start(featsT_sb[:], featsT_d[:])
        ident_sb = const.tile([128, 128], dt)
        nc.sync.dma_start(ident_sb[:], ident_d[:])
        masks_sb = const.tile([128, NT * 512], dt)
        for t in range(NT):
            nc.sync.dma_start(masks_sb[:, t * 512:(t + 1) * 512], mask_d[t])
        poolT_sb = const.tile([128, NT * 128], dt)
        for t in range(NT):
            nc.sync.dma_start(poolT_sb[:, t * 128:(t + 1) * 128], poolT_d[t])
        win_sb = const.tile([PF, D], dt)
        nc.sync.dma_start(win_sb[:], win_d[:])
        gfb_sb = const.tile([128, D], dt)
        nc.sync.dma_start(gfb_sb[:], gfb_d[:])
        bfrow_sb = const.tile([1, D], dt)
        nc.sync.dma_start(bfrow_sb[:], bfrow_d[:])
        rowsum_sb = const.tile([1, DOMCAP], dt)
        nc.sync.dma_start(rowsum_sb[:], rowsum_d[:])
        dlast_sb = const.tile([128, 2 * D], dt)
        for kt in range(2):
            nc.sync.dma_start(dlast_sb[:, kt * D:(kt + 1) * D], dlast_d[kt])
        blast_sb = const.tile([1, D], dt)
        nc.sync.dma_start(blast_sb[:], blast_d[:])
        ones_sb = const.tile([1, 512], dt)
        nc.gpsimd.memset(ones_sb[:], 1.0)
        eps_sb = const.tile([128, 1], dt)
        nc.gpsimd.memset(eps_sb[:], EPS)
        vone_sb = const.tile([128, NT * 264], dt)
        nc.gpsimd.memset(vone_sb[:], 1.0)

        def ln(src, dst):
            """dst = (src - mean(src)) * rsqrt(var(src) + eps) along free dim."""
            st6 = stat.tile([128, 6], dt, tag="st6")
            nc.vector.bn_stats(st6[:], src)
            mv = stat.tile([128, 2], dt, tag="mv")
            nc.vector.bn_aggr(mv[:], st6[:])
            lnv = stat.tile([128, 1], dt, tag="lnv")
            nc.scalar.activation(lnv[:], mv[:, 1:2], AF.Ln, bias=eps_sb[:])
            rstd = stat.tile([128, 1], dt, tag="rstd")
            nc.scalar.activation(rstd[:], lnv[:], AF.Exp, scale=-0.5)
            nc.vector.tensor_scalar(dst, src, mv[:, 0:1], rstd[:],
                                    op0=ALU.subtract, op1=ALU.mult)

        NCHUNKS = [(0, 512), (512, 128)]

        # ---- input embedding -> z0t [256, 640] (transposed stream)
        z0t = act.tile([128, 2 * NPAD], dt, tag="zstream")
        for c2 in range(2):
            for (n0, nn) in NCHUNKS:
                ps = ps_a.tile([128, 512], dt, tag="ps_a")
                MM(ps[:, :nn], win_sb[0:4, c2 * 128:(c2 + 1) * 128],
                   featsT_sb[0:4, n0:n0 + nn], start=True, stop=True)
                copy(z0t[:, c2 * NPAD + n0: c2 * NPAD + n0 + nn], ps[:, :nn])

        for l in range(L):
            # ---- per-layer weights (bufs=2 -> prefetch overlaps compute)
            wqk_sb = wpool.tile([128, 2 * 512], dt, tag="wqk")
            wv_sb = wpool.tile([128, 2 * 256], dt, tag="wv")
            wout_sb = wpool.tile([128, 2 * 256], dt, tag="wout")
            dprev_sb = wpool.tile([128, 2 * 256], dt, tag="dprev")
            w1_sb = wpool1.tile([128, 2 * 1024], dt, tag="w1")
            w2_sb = wpool1.tile([128, 8 * 256], dt, tag="w2")
            d1_sb = wpool.tile([128, 2 * 256], dt, tag="d1")
            for kt in range(2):
                nc.sync.dma_start(wqk_sb[:, kt * 512:(kt + 1) * 512], wqk_d[l, kt])
                nc.sync.dma_start(wv_sb[:, kt * 256:(kt + 1) * 256], wv_d[l, kt])
                nc.sync.dma_start(wout_sb[:, kt * 256:(kt + 1) * 256], wout_d[l, kt])
                nc.sync.dma_start(dprev_sb[:, kt * 256:(kt + 1) * 256], dprev_d[l, kt])
                nc.sync.dma_start(w1_sb[:, kt * 1024:(kt + 1) * 1024], w1_d[l, kt])
                nc.sync.dma_start(d1_sb[:, kt * 256:(kt + 1) * 256], d1_d[l, kt])
            for kt in range(8):
                nc.sync.dma_start(w2_sb[:, kt * 256:(kt + 1) * 256], w2_d[l, kt])
            bqk_sb = wpool.tile([1, 512], dt, tag="bqk")
            nc.sync.dma_start(bqk_sb[:], bqk_d[l])
            bv_sb = wpool.tile([1, 256], dt, tag="bv")
            nc.sync.dma_start(bv_sb[:], bv_d[l])
            bo2_sb = wpool.tile([1, 256], dt, tag="bo2")
            nc.sync.dma_start(bo2_sb[:], bo2_d[l])
            b1f_sb = wpool.tile([1, 1024], dt, tag="b1f")
            nc.sync.dma_start(b1f_sb[:], b1f_d[l])
            bf2_sb = wpool.tile([1, 256], dt, tag="bf2")
            nc.sync.dma_start(bf2_sb[:], bf2_d[l])

            # ---- Q^T,K^T [512, 640] -> qk_sb (4 chunks of [128, 640])
            qk_sb = act.tile([128, 4 * NPAD], dt, tag="qk")
            for m in range(4):
                for (n0, nn) in NCHUNKS:
                    ps = ps_a.tile([128, 512], dt, tag="ps_a")
                    MM(ps[:, :nn], bqk_sb[0:1, m * 128:(m + 1) * 128],
                       ones_sb[0:1, 0:nn], start=True, stop=False)
                    for kt in range(2):
                        MM(ps[:, :nn],
                           wqk_sb[:, kt * 512 + m * 128: kt * 512 + (m + 1) * 128],
                           z0t[:, kt * NPAD + n0: kt * NPAD + n0 + nn],
                           start=False, stop=(kt == 1))
                    copy(qk_sb[:, m * NPAD + n0: m * NPAD + n0 + nn], ps[:, :nn])

            # ---- V natural [640, 256] -> vone (strided, ones col at 32 of 33)
            for t in range(NT):
                ps = ps_s.tile([128, 264], dt, tag="ps_s")
                MM(ps[:, 0:256], ones_sb[0:1, 0:128], bv_sb[0:1, 0:256],
                   start=True, stop=False)
                for kt in range(2):
                    MM(ps[:, 0:256],
                       z0t[:, kt * NPAD + t * 128: kt * NPAD + (t + 1) * 128],
                       wv_sb[:, kt * 256:(kt + 1) * 256],
                       start=False, stop=(kt == 1))
                dst = vone_sb[:, t * 264:(t + 1) * 264] \
                    .rearrange("p (h c) -> p h c", c=33)[:, :, 0:32]
                src = ps[:, 0:256].rearrange("p (h c) -> p h c", c=32)
                nc.vector.tensor_copy(dst, src)

            # ---- attention per tile + a1 + LN1
            z1n = act.tile([128, NT * 256], dt, tag="z1n")
            for t in range(NT):
                pexp = act.tile([128, 1024], dt, tag="pexp")
                for g in range(2):
                    # one PSUM bank per head: concurrent row-group matmuls into
                    # the same bank hang the device
                    ps = ps_b.tile([128, 2048], dt, tag="ps4")
                    for hh in range(4):
                        h = g * 4 + hh
                        qc, qo = divmod(h * HD, 128)       # Q^T rows h*32..
                        kc, ko = divmod(D + h * HD, 128)   # K^T rows 256+h*32..
                        MM(ps[:, hh * 512: hh * 512 + 128],
                           qk_sb[ko:ko + HD,
                                 kc * NPAD + t * 128: kc * NPAD + (t + 1) * 128],
                           qk_sb[qo:qo + HD,
                                 qc * NPAD + t * 128: qc * NPAD + (t + 1) * 128],
                           start=True, stop=True, tile_position=(qo, 0))
                    nc.scalar.activation(
                        pexp[:, g * 512:(g + 1) * 512]
                            .rearrange("p (h c) -> p h c", c=128),
                        ps[:].rearrange("p (h c) -> p h c", c=512)[:, :, 0:128],
                        AF.Exp, scale=SCALE)
                pm = act.tile([128, 1024], dt, tag="pm")
                for g in range(2):
                    nc.gpsimd.tensor_tensor(pm[:, g * 512:(g + 1) * 512],
                                            pexp[:, g * 512:(g + 1) * 512],
                                            masks_sb[:, t * 512:(t + 1) * 512],
                                            op=ALU.mult)
                po = ps_s.tile([128, 264], dt, tag="ps_s")
                for h in range(8):
                    MM(po[:, h * 33:h * 33 + 33], pm[:, h * 128:(h + 1) * 128],
                       vone_sb[:, t * 264 + h * 33: t * 264 + h * 33 + 33],
                       start=True, stop=True)
                rec = stat.tile([128, 8], dt, tag="rec")
                nc.vector.reciprocal(
                    rec[:], po[:, 0:264].rearrange("p (h c) -> p h c", c=33)[:, :, 32:33])
                on = act.tile([128, 256], dt, tag="on")
                for h in range(8):
                    nc.vector.tensor_scalar_mul(on[:, h * 32:(h + 1) * 32],
                                                po[:, h * 33:h * 33 + 32],
                                                rec[:, h:h + 1])
                ot_t = act.tile([128, 256], dt, tag="ot")
                for c2 in range(2):
                    pst = ps_s.tile([128, 264], dt, tag="ps_s")
                    MM(pst[:, 0:128], on[:, c2 * 128:(c2 + 1) * 128], ident_sb[:],
                       start=True, stop=True)
                    copy(ot_t[:, c2 * 128:(c2 + 1) * 128], pst[:, 0:128])
                pa = ps_s.tile([128, 264], dt, tag="ps_s")
                MM(pa[:, 0:256], ones_sb[0:1, 0:128], bo2_sb[0:1, 0:256],
                   start=True, stop=False)
                for kt in range(2):
                    MM(pa[:, 0:256], ot_t[:, kt * 128:(kt + 1) * 128],
                       wout_sb[:, kt * 256:(kt + 1) * 256], start=False, stop=False)
                for kt in range(2):
                    MM(pa[:, 0:256],
                       z0t[:, kt * NPAD + t * 128: kt * NPAD + (t + 1) * 128],
                       dprev_sb[:, kt * 256:(kt + 1) * 256],
                       start=False, stop=(kt == 1))
                ln(pa[:, 0:256], z1n[:, t * 256:(t + 1) * 256])

            # ---- z1 transpose -> z1t [256, 640]
            z1t = act.tile([128, 2 * NPAD], dt, tag="z1t")
            for t in range(NT):
                for c2 in range(2):
                    pst = ps_s.tile([128, 264], dt, tag="ps_s")
                    MM(pst[:, 0:128], z1n[:, t * 256 + c2 * 128: t * 256 + (c2 + 1) * 128],
                       ident_sb[:], start=True, stop=True)
                    copy(z1t[:, c2 * NPAD + t * 128: c2 * NPAD + (t + 1) * 128],
                         pst[:, 0:128])

            # ---- FFN: hT = gelu(W1'^T z1t + b1') [1024, 640]
            ht = big.tile([128, 8 * NPAD], dt, tag="ht")
            for m in range(8):
                ph = ps_b.tile([128, 2048], dt, tag="ps4")
                for (n0, nn) in NCHUNKS:
                    MM(ph[:, n0:n0 + nn], b1f_sb[0:1, m * 128:(m + 1) * 128],
                       ones_sb[0:1, 0:nn], start=True, stop=False)
                    for kt in range(2):
                        MM(ph[:, n0:n0 + nn],
                           w1_sb[:, kt * 1024 + m * 128: kt * 1024 + (m + 1) * 128],
                           z1t[:, kt * NPAD + n0: kt * NPAD + n0 + nn],
                           start=False, stop=(kt == 1))
                nc.scalar.activation(ht[:, m * NPAD:(m + 1) * NPAD],
                                     ph[:, 0:NPAD], AF.Gelu)

            # ---- a2 + LN2 -> z2n
            z2n = act.tile([128, NT * 256], dt, tag="z2n")
            for t in range(NT):
                pa = ps_s.tile([128, 264], dt, tag="ps_s")
                MM(pa[:, 0:256], ones_sb[0:1, 0:128], bf2_sb[0:1, 0:256],
                   start=True, stop=False)
                for kt2 in range(8):
                    MM(pa[:, 0:256],
                       ht[:, kt2 * NPAD + t * 128: kt2 * NPAD + (t + 1) * 128],
                       w2_sb[:, kt2 * 256:(kt2 + 1) * 256], start=False, stop=False)
                for kt in range(2):
                    MM(pa[:, 0:256],
                       z1t[:, kt * NPAD + t * 128: kt * NPAD + (t + 1) * 128],
                       d1_sb[:, kt * 256:(kt + 1) * 256],
                       start=False, stop=(kt == 1))
                ln(pa[:, 0:256], z2n[:, t * 256:(t + 1) * 256])

            # ---- z2 transpose -> next z0t
            z0t = act.tile([128, 2 * NPAD], dt, tag="zstream")
            for t in range(NT):
                for c2 in range(2):
                    pst = ps_s.tile([128, 264], dt, tag="ps_s")
                    MM(pst[:, 0:128], z2n[:, t * 256 + c2 * 128: t * 256 + (c2 + 1) * 128],
                       ident_sb[:], start=True, stop=True)
                    copy(z0t[:, c2 * NPAD + t * 128: c2 * NPAD + (t + 1) * 128],
                         pst[:, 0:128])

        # ---- final LN + pooling
        xf = big.tile([128, NT * 256], dt, tag="xf")
        for t in range(NT):
            pa = ps_s.tile([128, 264], dt, tag="ps_s")
            MM(pa[:, 0:256], ones_sb[0:1, 0:128], blast_sb[0:1, 0:256],
               start=True, stop=False)
            for kt in range(2):
                MM(pa[:, 0:256],
                   z0t[:, kt * NPAD + t * 128: kt * NPAD + (t + 1) * 128],
                   dlast_sb[:, kt * 256:(kt + 1) * 256],
                   start=False, stop=(kt == 1))
            ln(pa[:, 0:256], xf[:, t * 256:(t + 1) * 256])

        pp = ps_s.tile([128, 264], dt, tag="ps_s")
        MM(pp[:, 0:256], rowsum_sb[0:1, 0:DOMCAP], bfrow_sb[0:1, 0:256],
           start=True, stop=False)
        for t in range(NT):
            MM(pp[:, 0:256], poolT_sb[:, t * 128:(t + 1) * 128],
               xf[:, t * 256:(t + 1) * 256], start=False, stop=(t == NT - 1))
        out_sb = act.tile([128, 256], dt, tag="outsb")
        nc.vector.tensor_tensor(out_sb[:], pp[:, 0:256], gfb_sb[:], op=ALU.mult)
        nc.sync.dma_start(out_d[:], out_sb[:])

    nc.compile()
    _CACHE["nc"] = nc
    return nc


# ------------------------------------------------------------------ interface


def kernel(**inputs):
    from concourse.bass_utils import run_bass_kernel_spmd

    ids = np.asarray(inputs["pulse_to_dom_idx"]).astype(np.int64)
    counts = np.bincount(ids, minlength=NDOM)
    cores = _pack_cores(ids)
    W = _fold_weights(inputs)
    ident = np.eye(128, dtype=np.float32)

    shared = dict(ident=ident, win=W["win"], wqk=W["wqk"], bqk=W["bqk"],
                  wv=W["wv"], bv=W["bv"], wout=W["wout"], dprev=W["dprev"],
                  bo2=W["bo2"], w1=W["w1"], b1f=W["b1f"], w2=W["w2"],
                  d1=W["d1"], bf2=W["bf2"], dlast=W["dlast"], blast=W["blast"],
                  gfb=W["gfb"], bfrow=W["bfrow"])

    in_maps = []
    for c in range(N_CORES):
        featsT, maskrep, poolT = _core_inputs(cores[c], inputs["pulse_features"], counts)
        rowsum = poolT.reshape(NPAD, DOMCAP).sum(0).reshape(1, DOMCAP)
        m = dict(shared)
        m.update(featsT=featsT, maskrep=maskrep, poolT=poolT,
                 rowsum=np.ascontiguousarray(rowsum, np.float32))
        in_maps.append(m)

    nc = _build_program()
    trace = os.environ.get("KERNEL_TRACE", "") not in ("", "0")
    res = run_bass_kernel_spmd(nc, in_maps, core_ids=list(range(N_CORES)),
                               trace=trace)
    if trace:
        kernel.last_exec_time_ns = res.exec_time_ns

    total = int(np.asarray(inputs["total_doms"]))
    out = np.zeros((total, D), np.float32)
    for c in range(N_CORES):
        ld = cores[c]["local_doms"]
        out[ld] = res.results[c]["out"][:len(ld)]
    return out


# revision 16
# speedup vs baseline: 1.6709x; 1.6709x over previous
"""DOMTransformer Trainium2 kernel (8 NeuronCores, SPMD single NEFF).

Strategy:
  - Pulses sharded across cores by whole DOMs; within a core DOMs are packed
    into 640 slots (5 tiles of 128) with no DOM crossing a 128-slot boundary,
    so attention reduces to 5 independent 128x128 block-diagonal tiles.
  - All LN affines, residual adds and biases are folded host-side into PE
    matmul accumulations (diag(gamma) as extra contraction blocks, biases as
    rank-1 ones x bias_row matmuls).
  - Attention in S^T = K^T.T @ Q^T orientation, one PSUM bank per head
    (concurrent row-group matmuls into one bank hang the device); softmax
    sums from an appended ones-column in the AV matmul.
  - LayerNorm rsqrt via DVE magic-constant Newton iteration (batched over the
    5 row tiles) - keeps the ACT engine free of table-set thrashing.
  - bf16 matmul operands everywhere except the input embedding and the final
    pooling (f32); PSUM accumulation is always f32.
"""
import os
import numpy as np
import ml_dtypes

BF16 = ml_dtypes.bfloat16

N_CORES = 8
N, PF, D, H, HD, L, DFF, NDOM = 4096, 4, 256, 8, 32, 4, 1024, 512
NPAD = 640
NT = NPAD // 128          # 5 tiles
DOMCAP = 128
EPS = 1e-5
SCALE = float(1.0 / np.sqrt(np.float32(HD)))
MAGIC = 0x5F3759DF

# ----------------------------------------------------------------- host side


def _pack_cores(pulse_to_dom_idx):
    ids = np.asarray(pulse_to_dom_idx).astype(np.int64)
    n = ids.shape[0]
    counts = np.bincount(ids, minlength=NDOM)
    doms = np.nonzero(counts)[0]
    dcounts = counts[doms]
    assert dcounts.max() <= 128, "dom larger than a tile is unsupported"
    starts = np.zeros(NDOM + 1, np.int64)
    starts[1:] = np.cumsum(counts)

    def packed_size(cnts):
        off = 0
        for k in cnts:
            if (off % 128) + k > 128:
                off = (off // 128 + 1) * 128
            off += k
        return off

    cores = []
    i = 0
    pulses_left = n
    for c in range(N_CORES):
        cores_left = N_CORES - c
        target = pulses_left / cores_left
        grp = []
        grp_pulses = 0
        while i < len(doms):
            k = dcounts[i]
            if grp and c < N_CORES - 1 and grp_pulses + k > target + 4:
                break
            if packed_size([dcounts[j] for j in range(i - len(grp), i + 1)]) > NPAD:
                break
            grp.append(i)
            grp_pulses += k
            i += 1
        cores.append(grp)
        pulses_left -= grp_pulses
    assert i == len(doms), f"packing overflow: {len(doms) - i} doms left over"

    out = []
    for grp in cores:
        slot_dom = np.full(NPAD, -1, np.int64)
        slot_pulse = np.full(NPAD, -1, np.int64)
        local_doms = []
        off = 0
        for gi in grp:
            d = doms[gi]
            k = dcounts[gi]
            if (off % 128) + k > 128:
                off = (off // 128 + 1) * 128
            slot_dom[off:off + k] = d
            slot_pulse[off:off + k] = np.arange(starts[d], starts[d] + k)
            local_doms.append(d)
            off += k
        assert len(local_doms) <= DOMCAP
        out.append(dict(slot_dom=slot_dom, slot_pulse=slot_pulse,
                        local_doms=np.array(local_doms, np.int64)))
    return out


def _core_inputs(core, pulse_features, counts):
    sd = core["slot_dom"].copy()
    sp = core["slot_pulse"]
    pad = sd < 0
    sd[pad] = NDOM + np.arange(NPAD)[pad]      # unique ids: pads attend to self

    feats = np.zeros((NPAD, PF), np.float32)
    feats[~pad] = np.asarray(pulse_features, np.float32)[sp[~pad]]
    featsT = np.ascontiguousarray(feats.T)

    maskrep = np.zeros((NT, 128, 512), BF16)
    for t in range(NT):
        s = sd[t * 128:(t + 1) * 128]
        m = (s[:, None] == s[None, :]).astype(BF16)
        maskrep[t] = np.tile(m, (1, 4))

    poolT = np.zeros((NT, 128, DOMCAP), np.float32)
    for li, d in enumerate(core["local_doms"]):
        sl = np.nonzero(core["slot_dom"] == d)[0]
        w = 1.0 / max(counts[d], 1)
        poolT[sl // 128, sl % 128, li] = w
    return featsT, maskrep, poolT


def _fold_weights(ins):
    f32 = lambda x: np.asarray(x, np.float32)
    Win, b_in = f32(ins["Win"]), f32(ins["b_in"])
    Wqkv, bqkv = f32(ins["Wqkv"]), f32(ins["bqkv"])
    Wout, bout = f32(ins["Wout"]), f32(ins["bout"])
    W1, b1 = f32(ins["W1"]), f32(ins["b1"])
    W2, b2 = f32(ins["W2"]), f32(ins["b2"])
    g1, be1 = f32(ins["g1"]), f32(ins["be1"])
    g2, be2 = f32(ins["g2"]), f32(ins["be2"])

    b = lambda a, shp: np.ascontiguousarray(np.asarray(a, np.float32).reshape(shp)).astype(BF16)
    W = {"win": np.ascontiguousarray(Win)}
    for nm in ("wqk", "bqk", "wv", "bv", "wout", "dprev", "bo2",
               "w1", "b1f", "w2", "d1", "bf2"):
        W[nm] = []
    for l in range(L):
        gp = np.ones(D, np.float32) if l == 0 else g2[l - 1]
        bp = b_in if l == 0 else be2[l - 1]
        Wf = Wqkv[l]
        W["wqk"].append(b(gp[:, None] * Wf[:, :2 * D], (2, 128, 512)))
        W["bqk"].append(b(bp @ Wf[:, :2 * D] + bqkv[l][:2 * D], (1, 512)))
        W["wv"].append(b(gp[:, None] * Wf[:, 2 * D:], (2, 128, 256)))
        W["bv"].append(b(bp @ Wf[:, 2 * D:] + bqkv[l][2 * D:], (1, 256)))
        W["wout"].append(b(Wout[l], (2, 128, 256)))
        W["dprev"].append(np.ascontiguousarray(np.diag(gp).reshape(2, 128, 256)))
        W["bo2"].append(b(bout[l] + bp, (1, 256)))
        W["w1"].append(b(g1[l][:, None] * W1[l], (2, 128, 1024)))
        W["b1f"].append(b(be1[l] @ W1[l] + b1[l], (1, 1024)))
        W["w2"].append(b(W2[l], (8, 128, 256)))
        W["d1"].append(np.ascontiguousarray(np.diag(g1[l]).reshape(2, 128, 256)))
        W["bf2"].append(b(b2[l] + be1[l], (1, 256)))
    for nm in ("wqk", "bqk", "wv", "bv", "wout", "dprev", "bo2",
               "w1", "b1f", "w2", "d1", "bf2"):
        W[nm] = np.stack(W[nm])
    W["dlast"] = np.ascontiguousarray(np.diag(g2[L - 1]).reshape(2, 128, 256).astype(np.float32))
    W["blast"] = b(be2[L - 1], (1, 256))
    W["gfb"] = np.ascontiguousarray(np.tile(f32(ins["gF"])[None, :], (128, 1)))
    W["bfrow"] = np.ascontiguousarray(f32(ins["bF"]).reshape(1, 256))
    return W


# ------------------------------------------------------------- device program

_CACHE = {}


def _build_program():
    if "nc" in _CACHE:
        return _CACHE["nc"]
    from contextlib import ExitStack
    import concourse.tile as tile
    import concourse.bacc as bacc
    import concourse.mybir as mybir

    dt = mybir.dt.float32
    bt = mybir.dt.bfloat16
    i32 = mybir.dt.int32
    AF = mybir.ActivationFunctionType
    ALU = mybir.AluOpType

    nc = bacc.Bacc("TRN2", target_bir_lowering=False, debug=False,
                   num_devices=N_CORES)

    def din(name, shape, dtype=dt):
        return nc.dram_tensor(name, shape, dtype, kind="ExternalInput").ap()

    featsT_d = din("featsT", (PF, NPAD))
    mask_d = din("maskrep", (NT, 128, 512), bt)
    poolT_d = din("poolT", (NT, 128, DOMCAP))
    ident_d = din("ident", (128, 128), bt)
    identf_d = din("identf", (128, 128))
    win_d = din("win", (PF, D))
    wqk_d = din("wqk", (L, 2, 128, 512), bt)
    bqk_d = din("bqk", (L, 1, 512), bt)
    wv_d = din("wv", (L, 2, 128, 256), bt)
    bv_d = din("bv", (L, 1, 256), bt)
    wout_d = din("wout", (L, 2, 128, 256), bt)
    dprev_d = din("dprev", (L, 2, 128, 256))
    bo2_d = din("bo2", (L, 1, 256), bt)
    w1_d = din("w1", (L, 2, 128, 1024), bt)
    b1f_d = din("b1f", (L, 1, 1024), bt)
    w2_d = din("w2", (L, 8, 128, 256), bt)
    d1_d = din("d1", (L, 2, 128, 256))
    bf2_d = din("bf2", (L, 1, 256), bt)
    dlast_d = din("dlast", (2, 128, 256))
    blast_d = din("blast", (1, 256), bt)
    gfb_d = din("gfb", (128, 256))
    bfrow_d = din("bfrow", (1, 256))
    rowsum_d = din("rowsum", (1, DOMCAP))
    out_d = nc.dram_tensor("out", (DOMCAP, D), dt, kind="ExternalOutput").ap()

    with tile.TileContext(nc) as tc, ExitStack() as ctx:
        const = ctx.enter_context(tc.tile_pool(name="const", bufs=1))
        wpool = ctx.enter_context(tc.tile_pool(name="w", bufs=2))
        act = ctx.enter_context(tc.tile_pool(name="act", bufs=2))
        big = ctx.enter_context(tc.tile_pool(name="big", bufs=1))
        stat = ctx.enter_context(tc.tile_pool(name="stat", bufs=6))
        ps_a = ctx.enter_context(tc.tile_pool(name="ps_a", bufs=2, space="PSUM"))
        ps_b = ctx.enter_context(tc.tile_pool(name="ps_b", bufs=1, space="PSUM"))
        ps_s = ctx.enter_context(tc.tile_pool(name="ps_s", bufs=2, space="PSUM"))

        MM = nc.tensor.matmul
        _cp = [0]

        def copy(dst, src):
            # alternate PSUM->SBUF copies between DVE and ACT
            if _cp[0] % 2 == 0:
                nc.vector.tensor_copy(dst, src)
            else:
                nc.scalar.copy(dst, src)
            _cp[0] += 1

        # ---- constants
        featsT_sb = const.tile([PF, NPAD], dt)
        nc.sync.dma_start(featsT_sb[:], featsT_d[:])
        ident_sb = const.tile([128, 128], bt)
        nc.sync.dma_start(ident_sb[:], ident_d[:])
        identf_sb = const.tile([128, 128], dt)
        nc.sync.dma_start(identf_sb[:], identf_d[:])
        masks_sb = const.tile([128, NT * 512], bt)
        for t in range(NT):
            nc.sync.dma_start(masks_sb[:, t * 512:(t + 1) * 512], mask_d[t])
        poolT_sb = const.tile([128, NT * 128], dt)
        for t in range(NT):
            nc.sync.dma_start(poolT_sb[:, t * 128:(t + 1) * 128], poolT_d[t])
        win_sb = const.tile([PF, D], dt)
        nc.sync.dma_start(win_sb[:], win_d[:])
        gfb_sb = const.tile([128, D], dt)
        nc.sync.dma_start(gfb_sb[:], gfb_d[:])
        bfrow_sb = const.tile([1, D], dt)
        nc.sync.dma_start(bfrow_sb[:], bfrow_d[:])
        rowsum_sb = const.tile([1, DOMCAP], dt)
        nc.sync.dma_start(rowsum_sb[:], rowsum_d[:])
        dlast_sb = const.tile([128, 2 * D], dt)
        for kt in range(2):
            nc.sync.dma_start(dlast_sb[:, kt * D:(kt + 1) * D], dlast_d[kt])
        blast_sb = const.tile([1, D], bt)
        nc.sync.dma_start(blast_sb[:], blast_d[:])
        ones_sb = const.tile([1, 640], bt)
        nc.gpsimd.memset(ones_sb[:], 1.0)
        vone_sb = const.tile([128, NT * 264], bt)
        nc.gpsimd.memset(vone_sb[:], 1.0)

        NCHUNKS = [(0, 512), (512, 128)]

        def ln_group(pa_list, dst, dst_dt_f32=False):
            """Batched LayerNorm over NT psum tiles.
            pa_list: list of (psum_ap,) accumulated pre-LN values [128,256].
            dst: SBUF tile [128, NT*256] for normalized output."""
            asb = act.tile([128, NT * 256], dt, tag="asb")
            mv10 = stat.tile([128, 2 * NT], dt, tag="mv10")
            for t, pa in enumerate(pa_list):
                copy(asb[:, t * 256:(t + 1) * 256], pa)
                st6 = stat.tile([128, 6], dt, tag="st6")
                nc.vector.bn_stats(st6[:], asb[:, t * 256:(t + 1) * 256])
                nc.vector.bn_aggr(mv10[:, 2 * t:2 * t + 2], st6[:])
            u5 = stat.tile([128, NT], dt, tag="u5")
            nc.vector.tensor_scalar(
                u5[:], mv10[:].rearrange("p (t two) -> p t two", two=2)[:, :, 1:2],
                EPS, None, op0=ALU.add)
            ti = stat.tile([128, NT], i32, tag="ti")
            nc.vector.tensor_scalar(ti[:], u5[:].bitcast(i32), 1, None,
                                    op0=ALU.arith_shift_right)
            y5 = stat.tile([128, NT], dt, tag="y5")
            nc.vector.tensor_scalar(y5[:].bitcast(i32), ti[:], -1, MAGIC,
                                    op0=ALU.mult, op1=ALU.add)
            h5 = stat.tile([128, NT], dt, tag="h5")
            for _ in range(2):
                nc.vector.tensor_tensor(h5[:], y5[:], y5[:], op=ALU.mult)
                nc.vector.tensor_tensor(h5[:], h5[:], u5[:], op=ALU.mult)
                nc.vector.tensor_scalar(h5[:], h5[:], -0.5, 1.5,
                                        op0=ALU.mult, op1=ALU.add)
                nc.vector.tensor_tensor(y5[:], y5[:], h5[:], op=ALU.mult)
            for t in range(NT):
                nc.vector.tensor_scalar(dst[:, t * 256:(t + 1) * 256],
                                        asb[:, t * 256:(t + 1) * 256],
                                        mv10[:, 2 * t:2 * t + 1],
                                        y5[:, t:t + 1],
                                        op0=ALU.subtract, op1=ALU.mult)

        # ---- input embedding -> z0t [256, 640] (transposed stream, bf16)
        z0t = act.tile([128, 2 * NPAD], dt, tag="zstream")
        z0tb = act.tile([128, 2 * NPAD], bt, tag="zstreamb")
        for c2 in range(2):
            for (n0, nn) in NCHUNKS:
                ps = ps_a.tile([128, 512], dt, tag="ps_a")
                MM(ps[:, :nn], win_sb[0:4, c2 * 128:(c2 + 1) * 128],
                   featsT_sb[0:4, n0:n0 + nn], start=True, stop=True)
                copy(z0t[:, c2 * NPAD + n0: c2 * NPAD + n0 + nn], ps[:, :nn])
                copy(z0tb[:, c2 * NPAD + n0: c2 * NPAD + n0 + nn], ps[:, :nn])

        for l in range(L):
            # ---- per-layer weights (bufs=2 -> prefetch overlaps compute)
            wqk_sb = wpool.tile([128, 2 * 512], bt, tag="wqk")
            wv_sb = wpool.tile([128, 2 * 256], bt, tag="wv")
            wout_sb = wpool.tile([128, 2 * 256], bt, tag="wout")
            dprev_sb = wpool.tile([128, 2 * 256], dt, tag="dprev")
            w1_sb = wpool.tile([128, 2 * 1024], bt, tag="w1")
            w2_sb = wpool.tile([128, 8 * 256], bt, tag="w2")
            d1_sb = wpool.tile([128, 2 * 256], dt, tag="d1")
            for kt in range(2):
                nc.sync.dma_start(wqk_sb[:, kt * 512:(kt + 1) * 512], wqk_d[l, kt])
                nc.sync.dma_start(wv_sb[:, kt * 256:(kt + 1) * 256], wv_d[l, kt])
                nc.sync.dma_start(wout_sb[:, kt * 256:(kt + 1) * 256], wout_d[l, kt])
                nc.sync.dma_start(dprev_sb[:, kt * 256:(kt + 1) * 256], dprev_d[l, kt])
                nc.sync.dma_start(w1_sb[:, kt * 1024:(kt + 1) * 1024], w1_d[l, kt])
                nc.sync.dma_start(d1_sb[:, kt * 256:(kt + 1) * 256], d1_d[l, kt])
            for kt in range(8):
                nc.sync.dma_start(w2_sb[:, kt * 256:(kt + 1) * 256], w2_d[l, kt])
            bqk_sb = wpool.tile([1, 512], bt, tag="bqk")
            nc.sync.dma_start(bqk_sb[:], bqk_d[l])
            bv_sb = wpool.tile([1, 256], bt, tag="bv")
            nc.sync.dma_start(bv_sb[:], bv_d[l])
            bo2_sb = wpool.tile([1, 256], bt, tag="bo2")
            nc.sync.dma_start(bo2_sb[:], bo2_d[l])
            b1f_sb = wpool.tile([1, 1024], bt, tag="b1f")
            nc.sync.dma_start(b1f_sb[:], b1f_d[l])
            bf2_sb = wpool.tile([1, 256], bt, tag="bf2")
            nc.sync.dma_start(bf2_sb[:], bf2_d[l])

            # ---- Q^T,K^T [512, 640] -> qk_sb (4 chunks of [128, 640], bf16)
            qk_sb = act.tile([128, 4 * NPAD], bt, tag="qk")
            for m in range(4):
                for (n0, nn) in NCHUNKS:
                    ps = ps_a.tile([128, 512], dt, tag="ps_a")
                    MM(ps[:, :nn], bqk_sb[0:1, m * 128:(m + 1) * 128],
                       ones_sb[0:1, 0:nn], start=True, stop=False)
                    for kt in range(2):
                        MM(ps[:, :nn],
                           wqk_sb[:, kt * 512 + m * 128: kt * 512 + (m + 1) * 128],
                           z0tb[:, kt * NPAD + n0: kt * NPAD + n0 + nn],
                           start=False, stop=(kt == 1))
                    copy(qk_sb[:, m * NPAD + n0: m * NPAD + n0 + nn], ps[:, :nn])

            # ---- V natural [640, 256] -> vone (strided, ones col at 32 of 33)
            for t in range(NT):
                ps = ps_s.tile([128, 264], dt, tag="ps_s")
                MM(ps[:, 0:256], ones_sb[0:1, 0:128], bv_sb[0:1, 0:256],
                   start=True, stop=False)
                for kt in range(2):
                    MM(ps[:, 0:256],
                       z0tb[:, kt * NPAD + t * 128: kt * NPAD + (t + 1) * 128],
                       wv_sb[:, kt * 256:(kt + 1) * 256],
                       start=False, stop=(kt == 1))
                dst = vone_sb[:, t * 264:(t + 1) * 264] \
                    .rearrange("p (h c) -> p h c", c=33)[:, :, 0:32]
                src = ps[:, 0:256].rearrange("p (h c) -> p h c", c=32)
                nc.vector.tensor_copy(dst, src)

            # ---- attention per tile + a1 accum
            pa1 = []
            for t in range(NT):
                pexp = act.tile([128, 1024], bt, tag="pexp")
                for g in range(2):
                    # one PSUM bank per head: concurrent row-group matmuls
                    # into the same bank hang the device
                    ps = ps_b.tile([128, 2048], dt, tag="ps4")
                    for hh in range(4):
                        h = g * 4 + hh
                        qc, qo = divmod(h * HD, 128)       # Q^T rows h*32..
                        kc, ko = divmod(D + h * HD, 128)   # K^T rows 256+h*32..
                        MM(ps[:, hh * 512: hh * 512 + 128],
                           qk_sb[ko:ko + HD,
                                 kc * NPAD + t * 128: kc * NPAD + (t + 1) * 128],
                           qk_sb[qo:qo + HD,
                                 qc * NPAD + t * 128: qc * NPAD + (t + 1) * 128],
                           start=True, stop=True, tile_position=(qo, 0))
                    nc.scalar.activation(
                        pexp[:, g * 512:(g + 1) * 512]
                            .rearrange("p (h c) -> p h c", c=128),
                        ps[:].rearrange("p (h c) -> p h c", c=512)[:, :, 0:128],
                        AF.Exp, scale=SCALE)
                pm = act.tile([128, 1024], bt, tag="pm")
                for g in range(2):
                    nc.gpsimd.tensor_tensor(pm[:, g * 512:(g + 1) * 512],
                                            pexp[:, g * 512:(g + 1) * 512],
                                            masks_sb[:, t * 512:(t + 1) * 512],
                                            op=ALU.mult)
                po = ps_s.tile([128, 264], dt, tag="ps_s")
                for h in range(8):
                    MM(po[:, h * 33:h * 33 + 33], pm[:, h * 128:(h + 1) * 128],
                       vone_sb[:, t * 264 + h * 33: t * 264 + h * 33 + 33],
                       start=True, stop=True)
                rec = stat.tile([128, 8], dt, tag="rec")
                nc.vector.reciprocal(
                    rec[:], po[:, 0:264].rearrange("p (h c) -> p h c", c=33)[:, :, 32:33])
                on = act.tile([128, 256], bt, tag="on")
                for h in range(8):
                    nc.vector.tensor_scalar_mul(on[:, h * 32:(h + 1) * 32],
                                                po[:, h * 33:h * 33 + 32],
                                                rec[:, h:h + 1])
                ot_t = act.tile([128, 256], bt, tag="ot")
                for c2 in range(2):
                    pst = ps_s.tile([128, 264], dt, tag="ps_s")
                    MM(pst[:, 0:128], on[:, c2 * 128:(c2 + 1) * 128], ident_sb[:],
                       start=True, stop=True)
                    copy(ot_t[:, c2 * 128:(c2 + 1) * 128], pst[:, 0:128])
                pa = ps_a.tile([128, 512], dt, tag="ps_a")
                MM(pa[:, 0:256], ones_sb[0:1, 0:128], bo2_sb[0:1, 0:256],
                   start=True, stop=False)
                for kt in range(2):
                    MM(pa[:, 0:256], ot_t[:, kt * 128:(kt + 1) * 128],
                       wout_sb[:, kt * 256:(kt + 1) * 256], start=False, stop=False)
                for kt in range(2):
                    MM(pa[:, 0:256],
                       z0t[:, kt * NPAD + t * 128: kt * NPAD + (t + 1) * 128],
                       dprev_sb[:, kt * 256:(kt + 1) * 256],
                       start=False, stop=(kt == 1))
                pa1.append(pa[:, 0:256])

            # ---- LN1 (batched) -> z1n bf16
            z1n = act.tile([128, NT * 256], dt, tag="z1n")
            ln_group(pa1, z1n)

            # ---- z1 transpose -> z1t [256, 640]
            z1t = act.tile([128, 2 * NPAD], dt, tag="z1t")
            z1tb = act.tile([128, 2 * NPAD], bt, tag="z1tb")
            for t in range(NT):
                for c2 in range(2):
                    pst = ps_s.tile([128, 264], dt, tag="ps_s")
                    MM(pst[:, 0:128], z1n[:, t * 256 + c2 * 128: t * 256 + (c2 + 1) * 128],
                       identf_sb[:], start=True, stop=True)
                    copy(z1t[:, c2 * NPAD + t * 128: c2 * NPAD + (t + 1) * 128],
                         pst[:, 0:128])
                    copy(z1tb[:, c2 * NPAD + t * 128: c2 * NPAD + (t + 1) * 128],
                         pst[:, 0:128])

            # ---- FFN: hT = gelu(W1'^T z1t + b1') [1024, 640] bf16
            ht = big.tile([128, 8 * NPAD], bt, tag="ht")
            for m in range(8):
                ph = ps_b.tile([128, 2048], dt, tag="ps4")
                for (n0, nn) in NCHUNKS:
                    MM(ph[:, n0:n0 + nn], b1f_sb[0:1, m * 128:(m + 1) * 128],
                       ones_sb[0:1, 0:nn], start=True, stop=False)
                    for kt in range(2):
                        MM(ph[:, n0:n0 + nn],
                           w1_sb[:, kt * 1024 + m * 128: kt * 1024 + (m + 1) * 128],
                           z1tb[:, kt * NPAD + n0: kt * NPAD + n0 + nn],
                           start=False, stop=(kt == 1))
                nc.scalar.activation(ht[:, m * NPAD:(m + 1) * NPAD],
                                     ph[:, 0:NPAD], AF.Gelu)

            # ---- a2 accum + LN2 -> z2n
            pa2 = []
            for t in range(NT):
                pa = ps_a.tile([128, 512], dt, tag="ps_a")
                MM(pa[:, 0:256], ones_sb[0:1, 0:128], bf2_sb[0:1, 0:256],
                   start=True, stop=False)
                for kt2 in range(8):
                    MM(pa[:, 0:256],
                       ht[:, kt2 * NPAD + t * 128: kt2 * NPAD + (t + 1) * 128],
                       w2_sb[:, kt2 * 256:(kt2 + 1) * 256], start=False, stop=False)
                for kt in range(2):
                    MM(pa[:, 0:256],
                       z1t[:, kt * NPAD + t * 128: kt * NPAD + (t + 1) * 128],
                       d1_sb[:, kt * 256:(kt + 1) * 256],
                       start=False, stop=(kt == 1))
                pa2.append(pa[:, 0:256])
            z2n = act.tile([128, NT * 256], dt, tag="z2n")
            ln_group(pa2, z2n)

            # ---- z2 transpose -> next z0t
            z0t = act.tile([128, 2 * NPAD], dt, tag="zstream")
            z0tb = act.tile([128, 2 * NPAD], bt, tag="zstreamb")
            for t in range(NT):
                for c2 in range(2):
                    pst = ps_s.tile([128, 264], dt, tag="ps_s")
                    MM(pst[:, 0:128], z2n[:, t * 256 + c2 * 128: t * 256 + (c2 + 1) * 128],
                       identf_sb[:], start=True, stop=True)
                    copy(z0t[:, c2 * NPAD + t * 128: c2 * NPAD + (t + 1) * 128],
                         pst[:, 0:128])
                    copy(z0tb[:, c2 * NPAD + t * 128: c2 * NPAD + (t + 1) * 128],
                         pst[:, 0:128])

        # ---- final LN + pooling (f32)
        paF = []
        for t in range(NT):
            pa = ps_a.tile([128, 512], dt, tag="ps_a")
            MM(pa[:, 0:256], ones_sb[0:1, 0:128], blast_sb[0:1, 0:256],
               start=True, stop=False)
            for kt in range(2):
                MM(pa[:, 0:256],
                   z0t[:, kt * NPAD + t * 128: kt * NPAD + (t + 1) * 128],
                   dlast_sb[:, kt * 256:(kt + 1) * 256],
                   start=False, stop=(kt == 1))
            paF.append(pa[:, 0:256])
        xf = big.tile([128, NT * 256], dt, tag="xf")
        # final LN normalizes into f32 (pooling stays f32)
        asbF = act.tile([128, NT * 256], dt, tag="asbF")
        mv10 = stat.tile([128, 2 * NT], dt, tag="mv10")
        for t, pa in enumerate(paF):
            copy(asbF[:, t * 256:(t + 1) * 256], pa)
            st6 = stat.tile([128, 6], dt, tag="st6")
            nc.vector.bn_stats(st6[:], asbF[:, t * 256:(t + 1) * 256])
            nc.vector.bn_aggr(mv10[:, 2 * t:2 * t + 2], st6[:])
        u5 = stat.tile([128, NT], dt, tag="u5")
        nc.vector.tensor_scalar(
            u5[:], mv10[:].rearrange("p (t two) -> p t two", two=2)[:, :, 1:2],
            EPS, None, op0=ALU.add)
        ti = stat.tile([128, NT], i32, tag="ti")
        nc.vector.tensor_scalar(ti[:], u5[:].bitcast(i32), 1, None,
                                op0=ALU.arith_shift_right)
        y5 = stat.tile([128, NT], dt, tag="y5")
        nc.vector.tensor_scalar(y5[:].bitcast(i32), ti[:], -1, MAGIC,
                                op0=ALU.mult, op1=ALU.add)
        h5 = stat.tile([128, NT], dt, tag="h5")
        for _ in range(2):
            nc.vector.tensor_tensor(h5[:], y5[:], y5[:], op=ALU.mult)
            nc.vector.tensor_tensor(h5[:], h5[:], u5[:], op=ALU.mult)
            nc.vector.tensor_scalar(h5[:], h5[:], -0.5, 1.5,
                                    op0=ALU.mult, op1=ALU.add)
            nc.vector.tensor_tensor(y5[:], y5[:], h5[:], op=ALU.mult)
        for t in range(NT):
            nc.vector.tensor_scalar(xf[:, t * 256:(t + 1) * 256],
                                    asbF[:, t * 256:(t + 1) * 256],
                                    mv10[:, 2 * t:2 * t + 1], y5[:, t:t + 1],
                                    op0=ALU.subtract, op1=ALU.mult)

        pp = ps_s.tile([128, 264], dt, tag="ps_s")
        MM(pp[:, 0:256], rowsum_sb[0:1, 0:DOMCAP], bfrow_sb[0:1, 0:256],
           start=True, stop=False)
        for t in range(NT):
            MM(pp[:, 0:256], poolT_sb[:, t * 128:(t + 1) * 128],
               xf[:, t * 256:(t + 1) * 256], start=False, stop=(t == NT - 1))
        out_sb = act.tile([128, 256], dt, tag="outsb")
        nc.vector.tensor_tensor(out_sb[:], pp[:, 0:256], gfb_sb[:], op=ALU.mult)
        nc.sync.dma_start(out_d[:], out_sb[:])

    nc.compile()
    _CACHE["nc"] = nc
    return nc


# ------------------------------------------------------------------ interface


def kernel(**inputs):
    from concourse.bass_utils import run_bass_kernel_spmd

    ids = np.asarray(inputs["pulse_to_dom_idx"]).astype(np.int64)
    counts = np.bincount(ids, minlength=NDOM)
    cores = _pack_cores(ids)
    W = _fold_weights(inputs)
    ident = np.eye(128, dtype=BF16)

    shared = dict(ident=ident, identf=np.eye(128, dtype=np.float32), win=W["win"], wqk=W["wqk"], bqk=W["bqk"],
                  wv=W["wv"], bv=W["bv"], wout=W["wout"], dprev=W["dprev"],
                  bo2=W["bo2"], w1=W["w1"], b1f=W["b1f"], w2=W["w2"],
                  d1=W["d1"], bf2=W["bf2"], dlast=W["dlast"], blast=W["blast"],
                  gfb=W["gfb"], bfrow=W["bfrow"])

    in_maps = []
    for c in range(N_CORES):
        featsT, maskrep, poolT = _core_inputs(cores[c], inputs["pulse_features"], counts)
        rowsum = poolT.reshape(NPAD, DOMCAP).sum(0).reshape(1, DOMCAP)
        m = dict(shared)
        m.update(featsT=featsT, maskrep=maskrep, poolT=poolT,
                 rowsum=np.ascontiguousarray(rowsum, np.float32))
        in_maps.append(m)

    nc = _build_program()
    trace = os.environ.get("KERNEL_TRACE", "") not in ("", "0")
    res = run_bass_kernel_spmd(nc, in_maps, core_ids=list(range(N_CORES)),
                               trace=trace)
    if trace:
        kernel.last_exec_time_ns = res.exec_time_ns

    total = int(np.asarray(inputs["total_doms"]))
    out = np.zeros((total, D), np.float32)
    for c in range(N_CORES):
        ld = cores[c]["local_doms"]
        out[ld] = res.results[c]["out"][:len(ld)]
    return out
